# revision 18
# baseline (speedup 1.0000x reference)
"""Trainium2 Bass kernel for a 2-layer DGCN (graph conv) on 8 NeuronCores.

Reference computation (fp32):
    h1  = relu(IFadj @ (x @ W1) + b1)         # [N, NHID]
    out = BN(adj @ (h1 @ W2) + b2)            # [N, OUTD], BN in eval mode

Distribution: rows of x / IFadj / adj are sharded across 8 cores
(row-parallel graph partitioning).

fp8 scheme (validated to rel_inf ~2.2e-3 vs fp32, same as bf16):
  The two N x N adjacency contractions run in fp8e4 (TRN FP8_EXP4 ==
  ml_dtypes.float8_e4m3) with DoubleRow perf mode (2 contraction rows
  per PE cell).  Direct fp8 quantization of the adjacency matrices is
  NOT accurate enough: the quantization bias of the activations (S)
  correlates with the nonzero mean (0.5) of the uniform adjacency
  entries and is then amplified ~4096x by layer 2.  Fix: mean-remove
  layer 1's adjacency on the host (D1 = IFadj - 0.5) and fold the exact
  rank-1 correction 0.5 * colsum(x @ W1) into b1 (host-side, free).
  Layer 2's quantization errors are benign without mean removal, so adj
  is cast to fp8 directly; W2 is pre-scaled by 0.5 so |Z| stays well
  inside fp8e4 range (the 2x is folded into the BN scale).

Per core k (rows R_k):
    phase 1: S_k = x[R_k] @ W1 (bf16 matmuls, fp8 evict) -> AllGather S
             (fp8, 2 pipelined chunks); plus a redundant S "tail" for
             the m-tiles {8k+4+q : k in 4..7} (identical on every core)
             so phase 2 has local work while the collective stream
             bootstraps (~60-100us of barrier+gather latency).
    phase 2: h1T = relu(S^T @ D1T + b1') in ONE fused pass holding all
             8 PSUM banks (4 jb x 2 ih), DoubleRow fp8 matmuls, group
             order: tail groups first, then gathered chunk 0, chunk 1.
    phase 3: z_k = h1T.T @ (0.5 W2)  (bf16) -> fp8 evict -> AllGather Z
             in 2 chunks.
    phase 4: outT = Z-as-lhsT vs adjT_k rhs (DoubleRow fp8), fused BN
             (with the 2x Z-rescale) in the PSUM-evict op.

The PE consumes the left operand transposed (out = lhsT.T @ rhs); the
host passes D1T / adjT / xT per core so no on-device transposes are
needed.  DoubleRow operands are [P, 2, free] slices pairing adjacent
m-tiles (contraction index m = t*128 + p).
"""

import numpy as np
import ml_dtypes

NCORES = 8
N = 8192
NFEAT = 1024
NHID = 512
OUTD = 256
ROWS = N // NCORES  # 1024
P = 128
BN_EPS = 1e-5

CB = NFEAT // P   # 8  c-blocks (x feature contraction)
IB = ROWS // P    # 8  i-blocks (local rows)
JB = NHID // P    # 4  j-blocks (hidden)
MT = N // P       # 64 m-tiles (global node contraction)
HF = 512          # matmul moving free dim (PSUM bank limit)
IH = ROWS // HF   # 2 i-halves of the local row range
OB = OUTD // P    # 2 output-feature blocks
GC = 2            # allgather chunks for both S and Z
QT = 4            # m-tiles per (core-block, chunk) = IB // GC
NPAIR = MT // 2   # 32 DoubleRow m-pairs over the full contraction
# tail: redundantly computed S rows, m-tiles {8k+4+q : k in TAILK}
TAILK = (2, 3, 4, 5, 6, 7)
NTB = 4 * len(TAILK)          # 24 tail i-blocks
TAIL_ROWS = NTB * P           # 3072
SPLIT_L = 8                   # trailing phase-2 groups run ih0-first

_BF16 = ml_dtypes.bfloat16
_FP8 = ml_dtypes.float8_e4m3  # TRN FP8_EXP4 (IEEE-ish, max +-240)

_cache = {}


def _build():
    import concourse.mybir as mybir
    import concourse.tile as tile
    from concourse import bacc

    dt = mybir.dt
    f32 = dt.float32
    bf16 = dt.bfloat16
    fp8 = dt.float8e4
    AF = mybir.ActivationFunctionType
    DR = mybir.MatmulPerfMode.DoubleRow

    nc = bacc.Bacc("TRN2", target_bir_lowering=False, debug=False,
                   num_devices=NCORES)

    xT_e = nc.dram_tensor("xT", [NFEAT, ROWS], fp8, kind="ExternalInput")
    xTt_e = nc.dram_tensor("xTt", [NFEAT, TAIL_ROWS], fp8,
                           kind="ExternalInput")
    ifadjT_e = nc.dram_tensor("ifadjT", [N, ROWS], fp8, kind="ExternalInput")
    adjT_e = nc.dram_tensor("adjT", [N, ROWS], fp8, kind="ExternalInput")
    w1_e = nc.dram_tensor("w1", [NFEAT, NHID], fp8, kind="ExternalInput")
    w2_e = nc.dram_tensor("w2", [NHID, OUTD], bf16, kind="ExternalInput")
    b1p_e = nc.dram_tensor("b1p", [P, JB], f32, kind="ExternalInput")
    bnsc_e = nc.dram_tensor("bnsc", [P, OB], f32, kind="ExternalInput")
    bnbi_e = nc.dram_tensor("bnbi", [P, OB], f32, kind="ExternalInput")
    # outT: [OUTD, ROWS]; the host transposes each core's block.
    out_e = nc.dram_tensor("out", [OUTD, ROWS], f32, kind="ExternalOutput")

    groups = [list(range(NCORES))]

    def allgather(g_in, g_out):
        nc.gpsimd.collective_compute(
            "AllGather", mybir.AluOpType.bypass, replica_groups=groups,
            ins=[g_in[:]], outs=[g_out[:]])

    with tile.TileContext(nc) as tc:
        with (
            tc.tile_pool(name="const", bufs=1) as const,
            tc.tile_pool(name="sloc", bufs=1) as sloc_p,
            tc.tile_pool(name="h1", bufs=1) as h1_p,
            tc.tile_pool(name="zsb", bufs=1) as z_p,
            tc.tile_pool(name="schunk", bufs=11) as schunk_p,
            tc.tile_pool(name="zchunk", bufs=10) as zchunk_p,
            tc.tile_pool(name="astream", bufs=16) as astream,
            tc.tile_pool(name="asplit", bufs=8) as asplit_p,
            tc.tile_pool(name="afull", bufs=10) as afull_p,
            tc.tile_pool(name="outsb", bufs=1) as outsb_p,
            tc.tile_pool(name="dram", bufs=1, space="DRAM") as dram,
        ):
            # ---- dummy tiny collective, first on the gpsimd queue: the
            # runtime's ncfw entry barrier precedes cc-op 0, so firing a
            # throwaway op at t~0 starts the ~35-50us barrier bootstrap
            # immediately instead of ~20us into the kernel.
            dum_in = dram.tile([1, 256], dt.uint8, name="dumin")
            dum_out = dram.tile([NCORES, 256], dt.uint8,
                                addr_space="Shared", name="dumout")
            allgather(dum_in, dum_out)

            # ---- constants into SBUF (xT/w1 first: phase 1 needs them)
            xT_sb = const.tile([P, CB, ROWS], fp8)
            nc.sync.dma_start(
                xT_sb[:], xT_e[:].rearrange("(cb p) i -> p cb i", p=P))
            w1_sb = const.tile([P, CB, NHID], fp8)
            nc.sync.dma_start(
                w1_sb[:], w1_e[:].rearrange("(cb p) j -> p cb j", p=P))
            xTt_sb = const.tile([P, CB, TAIL_ROWS], fp8)
            nc.sync.dma_start(
                xTt_sb[:], xTt_e[:].rearrange("(cb p) i -> p cb i", p=P))
            w2_sb = const.tile([P, JB, OUTD], bf16)
            nc.sync.dma_start(
                w2_sb[:], w2_e[:].rearrange("(jb p) o -> p jb o", p=P))
            b1p_sb = const.tile([P, JB], f32)
            nc.sync.dma_start(b1p_sb[:], b1p_e[:])
            bnsc_sb = const.tile([P, OB], f32)
            nc.sync.dma_start(bnsc_sb[:], bnsc_e[:])
            bnbi_sb = const.tile([P, OB], f32)
            nc.sync.dma_start(bnbi_sb[:], bnbi_e[:])

            # ---- DRAM bounce buffers for the chunked collectives
            RPC = ROWS // GC  # rows bounced per chunk (512)
            s_bounce = [dram.tile([RPC, NHID], fp8, name=f"sb{c}")
                        for c in range(GC)]
            s_all = [dram.tile([RPC * NCORES, NHID], fp8,
                               addr_space="Shared", name=f"sa{c}")
                     for c in range(GC)]
            z_bounce = [dram.tile([RPC, OUTD], fp8, name=f"zb{c}")
                        for c in range(GC)]
            z_all = [dram.tile([RPC * NCORES, OUTD], fp8,
                               addr_space="Shared", name=f"za{c}")
                     for c in range(GC)]

            # ---- phase 1: S_k = x[R_k] @ W1 (fp8 DoubleRow over feature
            # pairs); bounce + gather per chunk
            s_loc = sloc_p.tile([P, IB, NHID], fp8)
            IBC = IB // GC  # i-blocks per S chunk (4)
            CP = CB // 2    # DoubleRow feature-pair count (4)
            with tc.tile_pool(name="ps1", bufs=2, space="PSUM") as ps1:
                for c in range(GC):
                    for t in range(IBC):
                        ib = c * IBC + t
                        ps = ps1.tile([P, NHID], f32, tag="s")
                        for cp in range(CP):
                            nc.tensor.matmul(
                                ps[:],
                                xT_sb[:, 2 * cp:2 * cp + 2,
                                      ib * P:(ib + 1) * P],
                                w1_sb[:, 2 * cp:2 * cp + 2, :],
                                start=(cp == 0), stop=(cp == CP - 1),
                                perf_mode=DR,
                            )
                        nc.scalar.activation(s_loc[:, ib, :], ps[:], AF.Copy)
                        nc.sync.dma_start(
                            s_bounce[c][t * P:(t + 1) * P, :],
                            s_loc[:, ib, :])
                    allgather(s_bounce[c], s_all[c])
                # S-tail: every core redundantly computes S rows of
                # m-tiles {8k+4+q, k in TAILK} while the CC stream
                # bootstraps; phase 2 consumes these groups first.
                s_tail = sloc_p.tile([P, NTB, NHID], fp8)
                for tb in range(NTB):
                    ps = ps1.tile([P, NHID], f32, tag="s")
                    for cp in range(CP):
                        nc.tensor.matmul(
                            ps[:],
                            xTt_sb[:, 2 * cp:2 * cp + 2,
                                   tb * P:(tb + 1) * P],
                            w1_sb[:, 2 * cp:2 * cp + 2, :],
                            start=(cp == 0), stop=(cp == CP - 1),
                            perf_mode=DR,
                        )
                    nc.scalar.activation(s_tail[:, tb, :], ps[:], AF.Copy)

            # gathered-S chunk staging: chunk c, core-block k -> 4 m-rows
            # s_all[c] row (t p) j with t = 4k + q  <->  m-tile 8k + 4c + q
            s_sb = [[None] * NCORES for _ in range(GC)]

            def stage_s(c, k):
                # staged on the gpsimd queue: these DMAs wait on the
                # allgather's completion semaphore, and on the sync queue
                # they head-of-line block a_tile loads that are ready.
                tile_ = schunk_p.tile([P, QT, NHID], fp8, tag="schunk")
                nc.gpsimd.dma_start(
                    tile_[:],
                    s_all[c][k * QT * P:(k + 1) * QT * P, :]
                    .rearrange("(t p) j -> p t j", p=P))
                s_sb[c][k] = tile_

            h1T = h1_p.tile([P, JB, ROWS], bf16)
            z_sb = z_p.tile([P, IB, OUTD], fp8)

            # ---- phase 2: fused pass over all 64 m-tiles, both i-halves
            # accumulated in 8 PSUM banks, fp8 DoubleRow.  Group order:
            # locally-computed tail groups first (they need no gather),
            # then chunk 0 (gathered first), then chunk 1.  The trailing
            # SPLIT_L groups run ih=0 matmuls first so the z chunk 0
            # allgather fires ~10us before the last ih=1 matmul, hiding
            # the collective latency under remaining phase-2 + phase-4
            # lead-in work.  z matmuls reuse evicted psum_h banks.
            order = ([(1, k) for k in TAILK]
                     + [(0, k) for k in range(NCORES)]
                     + [(1, k) for k in range(NCORES) if k not in TAILK])
            fused_grps = order[:len(order) - SPLIT_L]
            split_grps = order[len(order) - SPLIT_L:]

            def pair_src(c, k, q2):
                if c == 1 and k in TAILK:
                    tb = 4 * TAILK.index(k) + 2 * q2
                    return s_tail[:, tb:tb + 2, :]
                return s_sb[c][k][:, 2 * q2:2 * q2 + 2, :]

            with tc.tile_pool(name="ps2", bufs=1, space="PSUM") as ps2:
                psum_h = [[ps2.tile([P, HF], f32, name=f"ph{jb}_{ih}",
                                    tag=f"ph{jb}_{ih}")
                           for ih in range(IH)] for jb in range(JB)]
                nmm = [0, 0]  # accumulated pairs per i-half

                def mm_pair(s_src, a_ap, ih):
                    for jb in range(JB):
                        nc.tensor.matmul(
                            psum_h[jb][ih][:],
                            s_src[:, :, jb * P:(jb + 1) * P],
                            a_ap,
                            start=(nmm[ih] == 0),
                            stop=(nmm[ih] == NPAIR - 1),
                            perf_mode=DR,
                        )
                    nmm[ih] += 1

                for c, k in fused_grps:
                    if not (c == 1 and k in TAILK):
                        stage_s(c, k)
                    for q2 in range(QT // 2):
                        mt = 8 * k + 4 * c + 2 * q2
                        a_tile = astream.tile([P, 2, ROWS], fp8, tag="apair")
                        nc.sync.dma_start(
                            a_tile[:],
                            ifadjT_e[mt * P:(mt + 2) * P, :]
                            .rearrange("(t p) i -> p t i", p=P))
                        # jb-major so consecutive matmuls share lhsT
                        src = pair_src(c, k, q2)
                        for jb in range(JB):
                            for ih in range(IH):
                                nc.tensor.matmul(
                                    psum_h[jb][ih][:],
                                    src[:, :, jb * P:(jb + 1) * P],
                                    a_tile[:, :, ih * HF:(ih + 1) * HF],
                                    start=(nmm[ih] == 0),
                                    stop=(nmm[ih] == NPAIR - 1),
                                    perf_mode=DR,
                                )
                        nmm[0] += 1
                        nmm[1] += 1
                for c, k in split_grps:
                    if not (c == 1 and k in TAILK):
                        stage_s(c, k)

                def split_half(ih):
                    # remaining contraction for i-half ih only
                    for c, k in split_grps:
                        for q2 in range(QT // 2):
                            mt = 8 * k + 4 * c + 2 * q2
                            a_half = asplit_p.tile([P, 2, HF], fp8,
                                                   tag="ahalf")
                            nc.sync.dma_start(
                                a_half[:],
                                ifadjT_e[mt * P:(mt + 2) * P,
                                         ih * HF:(ih + 1) * HF]
                                .rearrange("(t p) i -> p t i", p=P))
                            mm_pair(pair_src(c, k, q2), a_half[:], ih)
                    # relu+bias evict of this half, then z for its
                    # i-blocks (bf16), reusing the freed psum banks
                    for jb in range(JB):
                        nc.scalar.activation(
                            h1T[:, jb, ih * HF:(ih + 1) * HF],
                            psum_h[jb][ih][:], AF.Relu,
                            bias=b1p_sb[:, jb:jb + 1])
                    for t in range(IBC):
                        ib = ih * IBC + t
                        zps = psum_h[t][ih][:, :OUTD]
                        for jb in range(JB):
                            nc.tensor.matmul(
                                zps,
                                h1T[:, jb, ib * P:(ib + 1) * P],
                                w2_sb[:, jb, :],
                                start=(jb == 0), stop=(jb == JB - 1),
                            )
                        nc.scalar.activation(z_sb[:, ib, :], zps, AF.Copy)
                        nc.sync.dma_start(
                            z_bounce[ih][t * P:(t + 1) * P, :],
                            z_sb[:, ib, :])
                    allgather(z_bounce[ih], z_all[ih])

                for ih in range(IH):
                    split_half(ih)

            # ---- phase 4: outT[o, i] = sum_m Z[m, o] * adjT[m, i], fp8
            # DoubleRow, BN (with 2x Z-rescale) fused in the PSUM evict.
            outT_sb = outsb_p.tile([P, OB, ROWS], f32)
            with tc.tile_pool(name="ps4", bufs=1, space="PSUM") as ps4:
                psum_o = [[ps4.tile([P, HF], f32, name=f"po{ob}_{ih}",
                                    tag=f"po{ob}_{ih}")
                           for ih in range(IH)] for ob in range(OB)]
                first = True
                for c in range(GC):
                    for k in range(NCORES):
                        zc_sb = zchunk_p.tile([P, QT, OUTD], fp8,
                                              tag="zchunk")
                        nc.gpsimd.dma_start(
                            zc_sb[:],
                            z_all[c][k * QT * P:(k + 1) * QT * P, :]
                            .rearrange("(t p) o -> p t o", p=P))
                        final_grp = (c == GC - 1 and k == NCORES - 1)
                        if not final_grp:
                            for q2 in range(QT // 2):
                                mt = 8 * k + 4 * c + 2 * q2
                                a_tile = afull_p.tile([P, 2, ROWS], fp8,
                                                      tag="afull")
                                nc.sync.dma_start(
                                    a_tile[:],
                                    adjT_e[mt * P:(mt + 2) * P, :]
                                    .rearrange("(t p) i -> p t i", p=P))
                                for ob in range(OB):
                                    for ih in range(IH):
                                        nc.tensor.matmul(
                                            psum_o[ob][ih][:],
                                            zc_sb[:, 2 * q2:2 * q2 + 2,
                                                  ob * P:(ob + 1) * P],
                                            a_tile[:, :,
                                                   ih * HF:(ih + 1) * HF],
                                            start=first, stop=False,
                                            perf_mode=DR,
                                        )
                                first = False
                        else:
                            # last group: finish ob=0's accumulators first
                            # so their eviction overlaps ob=1's matmuls
                            a_tiles = []
                            for q2 in range(QT // 2):
                                mt = 8 * k + 4 * c + 2 * q2
                                a_tile = afull_p.tile([P, 2, ROWS], fp8,
                                                      tag="afull")
                                nc.sync.dma_start(
                                    a_tile[:],
                                    adjT_e[mt * P:(mt + 2) * P, :]
                                    .rearrange("(t p) i -> p t i", p=P))
                                a_tiles.append(a_tile)
                            for ob in range(OB):
                                for q2 in range(QT // 2):
                                    for ih in range(IH):
                                        nc.tensor.matmul(
                                            psum_o[ob][ih][:],
                                            zc_sb[:, 2 * q2:2 * q2 + 2,
                                                  ob * P:(ob + 1) * P],
                                            a_tiles[q2][:, :,
                                                        ih * HF:(ih + 1) * HF],
                                            start=False,
                                            stop=(q2 == QT // 2 - 1),
                                            perf_mode=DR,
                                        )
                # fused BN affine on PSUM evict: out = psum*scale + bias
                for ob in range(OB):
                    for ih in range(IH):
                        nc.vector.tensor_scalar(
                            outT_sb[:, ob, ih * HF:(ih + 1) * HF],
                            psum_o[ob][ih][:],
                            bnsc_sb[:, ob:ob + 1],
                            bnbi_sb[:, ob:ob + 1],
                            mybir.AluOpType.mult,
                            mybir.AluOpType.add)
                    nc.sync.dma_start(
                        out_e[ob * P:(ob + 1) * P, :], outT_sb[:, ob, :])

    nc.compile()
    return nc


def _get_nc():
    if "nc" not in _cache:
        _cache["nc"] = _build()
    return _cache["nc"]


def kernel(x, IFadj, adj, W1, b1, W2, b2, bn_gamma, bn_beta, bn_mean, bn_var):
    from concourse.bass_utils import run_bass_kernel_spmd

    x = np.asarray(x, dtype=np.float32)
    IFadj = np.asarray(IFadj, dtype=np.float32)
    adj = np.asarray(adj, dtype=np.float32)
    W1 = np.asarray(W1, dtype=np.float32)
    b1 = np.asarray(b1, dtype=np.float32)
    W2 = np.asarray(W2, dtype=np.float32)
    b2 = np.asarray(b2, dtype=np.float32)
    bn_gamma = np.asarray(bn_gamma, dtype=np.float32)
    bn_beta = np.asarray(bn_beta, dtype=np.float32)
    bn_mean = np.asarray(bn_mean, dtype=np.float32)
    bn_var = np.asarray(bn_var, dtype=np.float32)

    # host-side prep: shard rows, transpose for PE lhsT layout, cast.
    w1b = W1.astype(_FP8)
    w2b = (0.5 * W2).astype(_BF16)  # keep |Z| well inside fp8e4 range
    # b1' = b1 + 0.5 * colsum(x) @ W1  (exact rank-1 correction for the
    # mean-removed layer-1 adjacency)
    b1p_vec = b1 + 0.5 * (x.sum(axis=0, dtype=np.float64)
                          @ W1.astype(np.float64)).astype(np.float32)
    b1p = np.ascontiguousarray(b1p_vec.reshape(JB, P).T)  # [P, JB]
    inv = bn_gamma / np.sqrt(bn_var + BN_EPS)
    bias_tot = b2 * inv + bn_beta - bn_mean * inv
    bnsc = np.ascontiguousarray((2.0 * inv).reshape(OB, P).T)  # [P, OB]
    bnbi = np.ascontiguousarray(bias_tot.reshape(OB, P).T)     # [P, OB]

    # x rows of tail m-tiles {8k+4+q : k in TAILK}
    xTt = np.ascontiguousarray(np.concatenate(
        [x[(8 * k + 4) * P:(8 * k + 8) * P] for k in TAILK]).T).astype(_FP8)

    in_maps = []
    for k in range(NCORES):
        r0, r1 = k * ROWS, (k + 1) * ROWS
        in_maps.append({
            "xT": np.ascontiguousarray(x[r0:r1].T).astype(_FP8),
            "xTt": xTt,
            "ifadjT": (np.ascontiguousarray(IFadj[r0:r1].T)
                       - np.float32(0.5)).astype(_FP8),
            "adjT": np.ascontiguousarray(adj[r0:r1].T).astype(_FP8),
            "w1": w1b,
            "w2": w2b,
            "b1p": b1p,
            "bnsc": bnsc,
            "bnbi": bnbi,
        })

    global _last_in_maps
    _last_in_maps = in_maps

    nc = _get_nc()
    try:
        res = run_bass_kernel_spmd(nc, in_maps, list(range(NCORES)))
    except Exception:
        # transient device wedge (NRT_EXEC_UNIT_UNRECOVERABLE etc.) --
        # a straight retry has been observed to recover
        import time
        time.sleep(2.0)
        res = run_bass_kernel_spmd(nc, in_maps, list(range(NCORES)))
    # per-core output is outT [OUTD, ROWS]; transpose back and stack rows
    return np.concatenate(
        [np.ascontiguousarray(res.results[k]["out"].T)
         for k in range(NCORES)], axis=0)


# revision 20
# speedup vs baseline: 1.0698x; 1.0698x over previous
"""Trainium2 Bass kernel for a 2-layer DGCN (graph conv) on 8 NeuronCores.

Reference computation (fp32):
    h1  = relu(IFadj @ (x @ W1) + b1)         # [N, NHID]
    out = BN(adj @ (h1 @ W2) + b2)            # [N, OUTD], BN in eval mode

Distribution: rows of x / IFadj / adj are sharded across 8 cores
(row-parallel graph partitioning).

fp8 scheme (validated to rel_inf ~2.2e-3 vs fp32, same as bf16):
  The two N x N adjacency contractions run in fp8e4 (TRN FP8_EXP4 ==
  ml_dtypes.float8_e4m3) with DoubleRow perf mode (2 contraction rows
  per PE cell).  Direct fp8 quantization of the adjacency matrices is
  NOT accurate enough: the quantization bias of the activations (S)
  correlates with the nonzero mean (0.5) of the uniform adjacency
  entries and is then amplified ~4096x by layer 2.  Fix: mean-remove
  layer 1's adjacency on the host (D1 = IFadj - 0.5) and fold the exact
  rank-1 correction 0.5 * colsum(x @ W1) into b1 (host-side, free).
  Layer 2's quantization errors are benign without mean removal, so adj
  is cast to fp8 directly; W2 is pre-scaled by 0.5 so |Z| stays well
  inside fp8e4 range (the 2x is folded into the BN scale).

Per core k (rows R_k):
    phase 1: S_k = x[R_k] @ W1 (bf16 matmuls, fp8 evict) -> AllGather S
             (fp8, 2 pipelined chunks); plus a redundant S "tail" for
             the m-tiles {8k+4+q : k in 4..7} (identical on every core)
             so phase 2 has local work while the collective stream
             bootstraps (~60-100us of barrier+gather latency).
    phase 2: h1T = relu(S^T @ D1T + b1') in ONE fused pass holding all
             8 PSUM banks (4 jb x 2 ih), DoubleRow fp8 matmuls, group
             order: tail groups first, then gathered chunk 0, chunk 1.
    phase 3: z_k = h1T.T @ (0.5 W2)  (bf16) -> fp8 evict -> AllGather Z
             in 2 chunks.
    phase 4: outT = Z-as-lhsT vs adjT_k rhs (DoubleRow fp8), fused BN
             (with the 2x Z-rescale) in the PSUM-evict op.

The PE consumes the left operand transposed (out = lhsT.T @ rhs); the
host passes D1T / adjT / xT per core so no on-device transposes are
needed.  DoubleRow operands are [P, 2, free] slices pairing adjacent
m-tiles (contraction index m = t*128 + p).
"""

import numpy as np
import ml_dtypes

NCORES = 8
N = 8192
NFEAT = 1024
NHID = 512
OUTD = 256
ROWS = N // NCORES  # 1024
P = 128
BN_EPS = 1e-5

CB = NFEAT // P   # 8  c-blocks (x feature contraction)
IB = ROWS // P    # 8  i-blocks (local rows)
JB = NHID // P    # 4  j-blocks (hidden)
MT = N // P       # 64 m-tiles (global node contraction)
HF = 512          # matmul moving free dim (PSUM bank limit)
IH = ROWS // HF   # 2 i-halves of the local row range
OB = OUTD // P    # 2 output-feature blocks
GC = 2            # allgather chunks for both S and Z
QT = 4            # m-tiles per (core-block, chunk) = IB // GC
NPAIR = MT // 2   # 32 DoubleRow m-pairs over the full contraction
# tail: redundantly computed S rows, m-tiles {8k+4+q : k in TAILK}
TAILK = (2, 3, 4, 5, 6, 7)
NTB = 4 * len(TAILK)          # 24 tail i-blocks
TAIL_ROWS = NTB * P           # 3072
SPLIT_L = 8                   # trailing phase-2 groups run ih0-first

_BF16 = ml_dtypes.bfloat16
_FP8 = ml_dtypes.float8_e4m3  # TRN FP8_EXP4 (IEEE-ish, max +-240)

_cache = {}


def _build():
    import concourse.mybir as mybir
    import concourse.tile as tile
    from concourse import bacc

    dt = mybir.dt
    f32 = dt.float32
    bf16 = dt.bfloat16
    fp8 = dt.float8e4
    AF = mybir.ActivationFunctionType
    DR = mybir.MatmulPerfMode.DoubleRow

    nc = bacc.Bacc("TRN2", target_bir_lowering=False, debug=False,
                   num_devices=NCORES)

    xT_e = nc.dram_tensor("xT", [NFEAT, ROWS], fp8, kind="ExternalInput")
    xTt_e = nc.dram_tensor("xTt", [NFEAT, TAIL_ROWS], fp8,
                           kind="ExternalInput")
    ifadjT_e = nc.dram_tensor("ifadjT", [N, ROWS], fp8, kind="ExternalInput")
    adjT_e = nc.dram_tensor("adjT", [N, ROWS], fp8, kind="ExternalInput")
    w1_e = nc.dram_tensor("w1", [NFEAT, NHID], fp8, kind="ExternalInput")
    w2_e = nc.dram_tensor("w2", [NHID, OUTD], bf16, kind="ExternalInput")
    b1p_e = nc.dram_tensor("b1p", [P, JB], f32, kind="ExternalInput")
    bnsc_e = nc.dram_tensor("bnsc", [P, OB], f32, kind="ExternalInput")
    bnbi_e = nc.dram_tensor("bnbi", [P, OB], f32, kind="ExternalInput")
    # outT: [OUTD, ROWS]; the host transposes each core's block.
    out_e = nc.dram_tensor("out", [OUTD, ROWS], f32, kind="ExternalOutput")

    groups = [list(range(NCORES))]

    def allgather(g_in, g_out):
        nc.gpsimd.collective_compute(
            "AllGather", mybir.AluOpType.bypass, replica_groups=groups,
            ins=[g_in[:]], outs=[g_out[:]])

    with tile.TileContext(nc) as tc:
        with (
            tc.tile_pool(name="const", bufs=1) as const,
            tc.tile_pool(name="sloc", bufs=1) as sloc_p,
            tc.tile_pool(name="h1", bufs=1) as h1_p,
            tc.tile_pool(name="zsb", bufs=1) as z_p,
            tc.tile_pool(name="schunk", bufs=11) as schunk_p,
            tc.tile_pool(name="zchunk", bufs=10) as zchunk_p,
            tc.tile_pool(name="astream", bufs=16) as astream,
            tc.tile_pool(name="asplit", bufs=8) as asplit_p,
            tc.tile_pool(name="afull", bufs=16) as afull_p,
            tc.tile_pool(name="outsb", bufs=1) as outsb_p,
            tc.tile_pool(name="dram", bufs=1, space="DRAM") as dram,
        ):
            # ---- constants into SBUF (xT/w1 first: phase 1 needs them)
            xT_sb = const.tile([P, CB, ROWS], fp8)
            nc.sync.dma_start(
                xT_sb[:], xT_e[:].rearrange("(cb p) i -> p cb i", p=P))
            w1_sb = const.tile([P, CB, NHID], fp8)
            nc.sync.dma_start(
                w1_sb[:], w1_e[:].rearrange("(cb p) j -> p cb j", p=P))
            xTt_sb = const.tile([P, CB, TAIL_ROWS], fp8)
            nc.sync.dma_start(
                xTt_sb[:], xTt_e[:].rearrange("(cb p) i -> p cb i", p=P))
            w2_sb = const.tile([P, JB, OUTD], bf16)
            nc.sync.dma_start(
                w2_sb[:], w2_e[:].rearrange("(jb p) o -> p jb o", p=P))
            b1p_sb = const.tile([P, JB], f32)
            nc.sync.dma_start(b1p_sb[:], b1p_e[:])
            bnsc_sb = const.tile([P, OB], f32)
            nc.sync.dma_start(bnsc_sb[:], bnsc_e[:])
            bnbi_sb = const.tile([P, OB], f32)
            nc.sync.dma_start(bnbi_sb[:], bnbi_e[:])

            # ---- DRAM bounce buffers for the chunked collectives
            RPC = ROWS // GC  # rows bounced per chunk (512)
            s_bounce = [dram.tile([RPC, NHID], fp8, name=f"sb{c}")
                        for c in range(GC)]
            s_all = [dram.tile([RPC * NCORES, NHID], fp8,
                               addr_space="Shared", name=f"sa{c}")
                     for c in range(GC)]
            z_bounce = [dram.tile([RPC, OUTD], fp8, name=f"zb{c}")
                        for c in range(GC)]
            z_all = [dram.tile([RPC * NCORES, OUTD], fp8,
                               addr_space="Shared", name=f"za{c}")
                     for c in range(GC)]

            # ---- phase 1: S_k = x[R_k] @ W1 (fp8 DoubleRow over feature
            # pairs); bounce + gather per chunk
            s_loc = sloc_p.tile([P, IB, NHID], fp8)
            IBC = IB // GC  # i-blocks per S chunk (4)
            CP = CB // 2    # DoubleRow feature-pair count (4)
            with tc.tile_pool(name="ps1", bufs=2, space="PSUM") as ps1:
                for c in range(GC):
                    for t in range(IBC):
                        ib = c * IBC + t
                        ps = ps1.tile([P, NHID], f32, tag="s")
                        for cp in range(CP):
                            nc.tensor.matmul(
                                ps[:],
                                xT_sb[:, 2 * cp:2 * cp + 2,
                                      ib * P:(ib + 1) * P],
                                w1_sb[:, 2 * cp:2 * cp + 2, :],
                                start=(cp == 0), stop=(cp == CP - 1),
                                perf_mode=DR,
                            )
                        nc.scalar.activation(s_loc[:, ib, :], ps[:], AF.Copy)
                        nc.sync.dma_start(
                            s_bounce[c][t * P:(t + 1) * P, :],
                            s_loc[:, ib, :])
                    allgather(s_bounce[c], s_all[c])
                # S-tail: every core redundantly computes S rows of
                # m-tiles {8k+4+q, k in TAILK} while the CC stream
                # bootstraps; phase 2 consumes these groups first.
                s_tail = sloc_p.tile([P, NTB, NHID], fp8)
                for tb in range(NTB):
                    ps = ps1.tile([P, NHID], f32, tag="s")
                    for cp in range(CP):
                        nc.tensor.matmul(
                            ps[:],
                            xTt_sb[:, 2 * cp:2 * cp + 2,
                                   tb * P:(tb + 1) * P],
                            w1_sb[:, 2 * cp:2 * cp + 2, :],
                            start=(cp == 0), stop=(cp == CP - 1),
                            perf_mode=DR,
                        )
                    nc.scalar.activation(s_tail[:, tb, :], ps[:], AF.Copy)

            # gathered-S chunk staging: chunk c, core-block k -> 4 m-rows
            # s_all[c] row (t p) j with t = 4k + q  <->  m-tile 8k + 4c + q
            s_sb = [[None] * NCORES for _ in range(GC)]

            def stage_s(c, k):
                # staged on the gpsimd queue: these DMAs wait on the
                # allgather's completion semaphore, and on the sync queue
                # they head-of-line block a_tile loads that are ready.
                tile_ = schunk_p.tile([P, QT, NHID], fp8, tag="schunk")
                nc.gpsimd.dma_start(
                    tile_[:],
                    s_all[c][k * QT * P:(k + 1) * QT * P, :]
                    .rearrange("(t p) j -> p t j", p=P))
                s_sb[c][k] = tile_

            h1T = h1_p.tile([P, JB, ROWS], bf16)
            z_sb = z_p.tile([P, IB, OUTD], fp8)

            # ---- phase 2: fused pass over all 64 m-tiles, both i-halves
            # accumulated in 8 PSUM banks, fp8 DoubleRow.  Group order:
            # locally-computed tail groups first (they need no gather),
            # then chunk 0 (gathered first), then chunk 1.  The trailing
            # SPLIT_L groups run ih=0 matmuls first so the z chunk 0
            # allgather fires ~10us before the last ih=1 matmul, hiding
            # the collective latency under remaining phase-2 + phase-4
            # lead-in work.  z matmuls reuse evicted psum_h banks.
            order = ([(1, k) for k in TAILK]
                     + [(0, k) for k in range(NCORES)]
                     + [(1, k) for k in range(NCORES) if k not in TAILK])
            fused_grps = order[:len(order) - SPLIT_L]
            split_grps = order[len(order) - SPLIT_L:]

            def pair_src(c, k, q2):
                if c == 1 and k in TAILK:
                    tb = 4 * TAILK.index(k) + 2 * q2
                    return s_tail[:, tb:tb + 2, :]
                return s_sb[c][k][:, 2 * q2:2 * q2 + 2, :]

            with tc.tile_pool(name="ps2", bufs=1, space="PSUM") as ps2:
                psum_h = [[ps2.tile([P, HF], f32, name=f"ph{jb}_{ih}",
                                    tag=f"ph{jb}_{ih}")
                           for ih in range(IH)] for jb in range(JB)]
                nmm = [0, 0]  # accumulated pairs per i-half

                def mm_pair(s_src, a_ap, ih):
                    for jb in range(JB):
                        nc.tensor.matmul(
                            psum_h[jb][ih][:],
                            s_src[:, :, jb * P:(jb + 1) * P],
                            a_ap,
                            start=(nmm[ih] == 0),
                            stop=(nmm[ih] == NPAIR - 1),
                            perf_mode=DR,
                        )
                    nmm[ih] += 1

                for c, k in fused_grps:
                    if not (c == 1 and k in TAILK):
                        stage_s(c, k)
                    for q2 in range(QT // 2):
                        mt = 8 * k + 4 * c + 2 * q2
                        a_tile = astream.tile([P, 2, ROWS], fp8, tag="apair")
                        nc.sync.dma_start(
                            a_tile[:],
                            ifadjT_e[mt * P:(mt + 2) * P, :]
                            .rearrange("(t p) i -> p t i", p=P))
                        # jb-major so consecutive matmuls share lhsT
                        src = pair_src(c, k, q2)
                        for jb in range(JB):
                            for ih in range(IH):
                                nc.tensor.matmul(
                                    psum_h[jb][ih][:],
                                    src[:, :, jb * P:(jb + 1) * P],
                                    a_tile[:, :, ih * HF:(ih + 1) * HF],
                                    start=(nmm[ih] == 0),
                                    stop=(nmm[ih] == NPAIR - 1),
                                    perf_mode=DR,
                                )
                        nmm[0] += 1
                        nmm[1] += 1
                for c, k in split_grps:
                    if not (c == 1 and k in TAILK):
                        stage_s(c, k)

                def split_half(ih):
                    # remaining contraction for i-half ih only
                    for c, k in split_grps:
                        for q2 in range(QT // 2):
                            mt = 8 * k + 4 * c + 2 * q2
                            a_half = asplit_p.tile([P, 2, HF], fp8,
                                                   tag="ahalf")
                            nc.sync.dma_start(
                                a_half[:],
                                ifadjT_e[mt * P:(mt + 2) * P,
                                         ih * HF:(ih + 1) * HF]
                                .rearrange("(t p) i -> p t i", p=P))
                            mm_pair(pair_src(c, k, q2), a_half[:], ih)
                    # relu+bias evict of this half, then z for its
                    # i-blocks (bf16), reusing the freed psum banks
                    for jb in range(JB):
                        nc.scalar.activation(
                            h1T[:, jb, ih * HF:(ih + 1) * HF],
                            psum_h[jb][ih][:], AF.Relu,
                            bias=b1p_sb[:, jb:jb + 1])
                    for t in range(IBC):
                        ib = ih * IBC + t
                        zps = psum_h[t][ih][:, :OUTD]
                        for jb in range(JB):
                            nc.tensor.matmul(
                                zps,
                                h1T[:, jb, ib * P:(ib + 1) * P],
                                w2_sb[:, jb, :],
                                start=(jb == 0), stop=(jb == JB - 1),
                            )
                        nc.scalar.activation(z_sb[:, ib, :], zps, AF.Copy)
                        nc.sync.dma_start(
                            z_bounce[ih][t * P:(t + 1) * P, :],
                            z_sb[:, ib, :])
                    allgather(z_bounce[ih], z_all[ih])

                for ih in range(IH):
                    split_half(ih)

            # ---- phase 4: outT[o, i] = sum_m Z[m, o] * adjT[m, i], fp8
            # DoubleRow, BN (with 2x Z-rescale) fused in the PSUM evict.
            outT_sb = outsb_p.tile([P, OB, ROWS], f32)
            with tc.tile_pool(name="ps4", bufs=1, space="PSUM") as ps4:
                psum_o = [[ps4.tile([P, HF], f32, name=f"po{ob}_{ih}",
                                    tag=f"po{ob}_{ih}")
                           for ih in range(IH)] for ob in range(OB)]
                first = True
                for c in range(GC):
                    for k in range(NCORES):
                        zc_sb = zchunk_p.tile([P, QT, OUTD], fp8,
                                              tag="zchunk")
                        nc.gpsimd.dma_start(
                            zc_sb[:],
                            z_all[c][k * QT * P:(k + 1) * QT * P, :]
                            .rearrange("(t p) o -> p t o", p=P))
                        final_grp = (c == GC - 1 and k == NCORES - 1)
                        if not final_grp:
                            for q2 in range(QT // 2):
                                mt = 8 * k + 4 * c + 2 * q2
                                a_tile = afull_p.tile([P, 2, ROWS], fp8,
                                                      tag="afull")
                                nc.sync.dma_start(
                                    a_tile[:],
                                    adjT_e[mt * P:(mt + 2) * P, :]
                                    .rearrange("(t p) i -> p t i", p=P))
                                for ob in range(OB):
                                    for ih in range(IH):
                                        nc.tensor.matmul(
                                            psum_o[ob][ih][:],
                                            zc_sb[:, 2 * q2:2 * q2 + 2,
                                                  ob * P:(ob + 1) * P],
                                            a_tile[:, :,
                                                   ih * HF:(ih + 1) * HF],
                                            start=first, stop=False,
                                            perf_mode=DR,
                                        )
                                first = False
                        else:
                            # last group: finish ob=0's accumulators first
                            # so their eviction overlaps ob=1's matmuls
                            a_tiles = []
                            for q2 in range(QT // 2):
                                mt = 8 * k + 4 * c + 2 * q2
                                a_tile = afull_p.tile([P, 2, ROWS], fp8,
                                                      tag="afull")
                                nc.sync.dma_start(
                                    a_tile[:],
                                    adjT_e[mt * P:(mt + 2) * P, :]
                                    .rearrange("(t p) i -> p t i", p=P))
                                a_tiles.append(a_tile)
                            for ob in range(OB):
                                for q2 in range(QT // 2):
                                    for ih in range(IH):
                                        nc.tensor.matmul(
                                            psum_o[ob][ih][:],
                                            zc_sb[:, 2 * q2:2 * q2 + 2,
                                                  ob * P:(ob + 1) * P],
                                            a_tiles[q2][:, :,
                                                        ih * HF:(ih + 1) * HF],
                                            start=False,
                                            stop=(q2 == QT // 2 - 1),
                                            perf_mode=DR,
                                        )
                # fused BN affine on PSUM evict: out = psum*scale + bias
                for ob in range(OB):
                    for ih in range(IH):
                        nc.vector.tensor_scalar(
                            outT_sb[:, ob, ih * HF:(ih + 1) * HF],
                            psum_o[ob][ih][:],
                            bnsc_sb[:, ob:ob + 1],
                            bnbi_sb[:, ob:ob + 1],
                            mybir.AluOpType.mult,
                            mybir.AluOpType.add)
                    nc.sync.dma_start(
                        out_e[ob * P:(ob + 1) * P, :], outT_sb[:, ob, :])

    nc.compile()
    return nc


def _get_nc():
    if "nc" not in _cache:
        _cache["nc"] = _build()
    return _cache["nc"]


def kernel(x, IFadj, adj, W1, b1, W2, b2, bn_gamma, bn_beta, bn_mean, bn_var):
    from concourse.bass_utils import run_bass_kernel_spmd

    x = np.asarray(x, dtype=np.float32)
    IFadj = np.asarray(IFadj, dtype=np.float32)
    adj = np.asarray(adj, dtype=np.float32)
    W1 = np.asarray(W1, dtype=np.float32)
    b1 = np.asarray(b1, dtype=np.float32)
    W2 = np.asarray(W2, dtype=np.float32)
    b2 = np.asarray(b2, dtype=np.float32)
    bn_gamma = np.asarray(bn_gamma, dtype=np.float32)
    bn_beta = np.asarray(bn_beta, dtype=np.float32)
    bn_mean = np.asarray(bn_mean, dtype=np.float32)
    bn_var = np.asarray(bn_var, dtype=np.float32)

    # host-side prep: shard rows, transpose for PE lhsT layout, cast.
    w1b = W1.astype(_FP8)
    w2b = (0.5 * W2).astype(_BF16)  # keep |Z| well inside fp8e4 range
    # b1' = b1 + 0.5 * colsum(x) @ W1  (exact rank-1 correction for the
    # mean-removed layer-1 adjacency)
    b1p_vec = b1 + 0.5 * (x.sum(axis=0, dtype=np.float64)
                          @ W1.astype(np.float64)).astype(np.float32)
    b1p = np.ascontiguousarray(b1p_vec.reshape(JB, P).T)  # [P, JB]
    inv = bn_gamma / np.sqrt(bn_var + BN_EPS)
    bias_tot = b2 * inv + bn_beta - bn_mean * inv
    bnsc = np.ascontiguousarray((2.0 * inv).reshape(OB, P).T)  # [P, OB]
    bnbi = np.ascontiguousarray(bias_tot.reshape(OB, P).T)     # [P, OB]

    # x rows of tail m-tiles {8k+4+q : k in TAILK}
    xTt = np.ascontiguousarray(np.concatenate(
        [x[(8 * k + 4) * P:(8 * k + 8) * P] for k in TAILK]).T).astype(_FP8)

    in_maps = []
    for k in range(NCORES):
        r0, r1 = k * ROWS, (k + 1) * ROWS
        in_maps.append({
            "xT": np.ascontiguousarray(x[r0:r1].T).astype(_FP8),
            "xTt": xTt,
            "ifadjT": (np.ascontiguousarray(IFadj[r0:r1].T)
                       - np.float32(0.5)).astype(_FP8),
            "adjT": np.ascontiguousarray(adj[r0:r1].T).astype(_FP8),
            "w1": w1b,
            "w2": w2b,
            "b1p": b1p,
            "bnsc": bnsc,
            "bnbi": bnbi,
        })

    global _last_in_maps
    _last_in_maps = in_maps

    nc = _get_nc()
    try:
        res = run_bass_kernel_spmd(nc, in_maps, list(range(NCORES)))
    except Exception:
        # transient device wedge (NRT_EXEC_UNIT_UNRECOVERABLE etc.) --
        # a straight retry has been observed to recover
        import time
        time.sleep(2.0)
        res = run_bass_kernel_spmd(nc, in_maps, list(range(NCORES)))
    # per-core output is outT [OUTD, ROWS]; transpose back and stack rows
    return np.concatenate(
        [np.ascontiguousarray(res.results[k]["out"].T)
         for k in range(NCORES)], axis=0)
